# revision 1
# baseline (speedup 1.0000x reference)
"""DSSM (dual GRU encoder + BxB softmax similarity) on 8 Trainium2 NeuronCores.

Strategy:
  - Cores 0-3 run the context encoder on batch rows [256c, 256(c+1));
    cores 4-7 run the reply encoder on rows [256(c-4), 256(c-3)).
    One SPMD program; per-core in_maps carry the right table/weights/indices.
  - Encoder: backward GRU over L=128 steps, hidden state kept transposed
    (features on partitions): h as [128 part, 4 chunk, 256 batch] fp16.
    Per step: 66 fp16 matmuls (gi K=101 incl. folded biases via ones column,
    gh K=128, bhh_n via K=1 ones outer product) accumulate gate
    pre-activations in PSUM; ACT does sigmoid/tanh; DVE combines.
    Embedding rows arrive via per-step indirect-DMA gathers (+ PE transpose).
  - Phase 2 (second small kernel): each core computes a 128-row slice of
    scores = hc @ hr.T (fp16 matmuls, fp32 accum) + row softmax.

All tensor layout prep (transposes, bias folding, sharding, time reversal)
is host-side numpy; the FLOP-carrying work runs on device.
"""

import numpy as np

import concourse.bass as bass
import concourse.mybir as mybir
import concourse.tile as tile
from concourse import bacc
from concourse.bass_utils import run_bass_kernel_spmd
from concourse.masks import make_identity

F16 = mybir.dt.float16
F32 = mybir.dt.float32
I32 = mybir.dt.int32

V, E, H, B, L = 50000, 100, 512, 1024, 128
NB = 256          # batch rows per core
HC = H // 128     # 4 h chunks
GA = 6            # gather lookahead (steps)
XA = 2            # xt lookahead


def build_encoder(l_steps=L, loop_n=None):
    """loop_n: benchmark-only mode — wraps the step loop in a For_i that
    re-runs the whole sequence loop_n times (data goes stale; timing only)."""
    nc = bacc.Bacc("TRN2", target_bir_lowering=False, debug=False)
    emb_d = nc.dram_tensor("emb", [V, E], F16, kind="ExternalInput")
    idx_d = nc.dram_tensor("idx", [128, 2 * l_steps], I32, kind="ExternalInput")
    wih_d = nc.dram_tensor("wih", [E + 1, 12, 128], F16, kind="ExternalInput")
    whh_d = nc.dram_tensor("whh", [128, HC, 3 * H], F16, kind="ExternalInput")
    bhhn_d = nc.dram_tensor("bhhn", [1, H], F16, kind="ExternalInput")
    hout_d = nc.dram_tensor("hout", [128, HC, NB], F16, kind="ExternalOutput")

    with tile.TileContext(nc) as tc:
        with (
            tc.tile_pool(name="wt", bufs=1) as wt,
            tc.tile_pool(name="grng", bufs=GA + 2) as grng,
            tc.tile_pool(name="xrng", bufs=XA + 2) as xrng,
            tc.tile_pool(name="hrng", bufs=3) as hrng,
            tc.tile_pool(name="gt", bufs=4) as gt,
            # one PSUM pool per bank; pstr also hosts gin0 (time-shared slot)
            tc.tile_pool(name="psra", bufs=1, space="PSUM") as psra,
            tc.tile_pool(name="psrb", bufs=1, space="PSUM") as psrb,
            tc.tile_pool(name="psza", bufs=1, space="PSUM") as psza,
            tc.tile_pool(name="pszb", bufs=1, space="PSUM") as pszb,
            tc.tile_pool(name="psga", bufs=1, space="PSUM") as psga,
            tc.tile_pool(name="psgb", bufs=1, space="PSUM") as psgb,
            tc.tile_pool(name="psgin", bufs=1, space="PSUM") as psgin,
            tc.tile_pool(name="pstr", bufs=1, space="PSUM") as pstr,
        ):
            # --- constants / weights (idx first: gathers need it; whh last:
            # step 0 skips its gh matmuls since h0 == 0, so whh is only
            # needed from step 1) ---
            idx_t = wt.tile([128, 2 * l_steps], I32, tag="idx")
            nc.sync.dma_start(out=idx_t[:], in_=idx_d[:])
            wih_t = wt.tile([E + 1, 12, 128], F16, tag="wih")
            nc.sync.dma_start(out=wih_t[:], in_=wih_d[:])
            bhhn_t = wt.tile([1, H], F16, tag="bhhn")
            nc.sync.dma_start(out=bhhn_t[:], in_=bhhn_d[:])
            # whh split r/n/z in consumption order: step 1's r matmuls can
            # start ~3us before the z columns finish loading
            whh_t = wt.tile([128, HC, 3 * H], F16, tag="whh")
            nc.sync.dma_start(out=whh_t[:, :, 0 : H], in_=whh_d[:, :, 0 : H])
            nc.sync.dma_start(out=whh_t[:, :, 2 * H : 3 * H], in_=whh_d[:, :, 2 * H : 3 * H])
            nc.sync.dma_start(out=whh_t[:, :, H : 2 * H], in_=whh_d[:, :, H : 2 * H])
            ident = wt.tile([128, 128], F16, tag="ident")
            make_identity(nc, ident[:])
            ones_t = wt.tile([1, NB], F16, tag="ones")
            nc.vector.memset(ones_t[:], 1.0)

            # --- rings ---
            n_g = GA + 2
            g_ring = []
            for i in range(n_g):
                g = grng.tile([128, 2, E + 1], F16, tag="g", name=f"g{i}")
                nc.vector.memset(g[:, :, E : E + 1], 1.0)
                g_ring.append(g)
            n_x = XA + 2
            xt_ring = [xrng.tile([E + 1, NB], F16, tag="xt", name=f"xt{i}") for i in range(n_x)]
            # no h memsets: step 0 skips all gh matmuls and computes its hmn
            # as -n directly, so no slot is ever read before it is written
            h_ring = [hrng.tile([128, HC, NB], F16, tag="h", name=f"h{i}")
                      for i in range(3)]

            def emit_gather(s):
                g = g_ring[s % n_g]
                for hh in range(2):
                    nc.gpsimd.indirect_dma_start(
                        out=g[:, hh, :E],
                        out_offset=None,
                        in_=emb_d[:],
                        in_offset=bass.IndirectOffsetOnAxis(
                            ap=idx_t[:, 2 * s + hh : 2 * s + hh + 1], axis=0
                        ),
                    )

            pts = {}

            def emit_xt_tr(s):
                g = g_ring[s % n_g]
                pt = pstr.tile([E + 1, 2, 128], F16, tag="pt", name=f"pt{s}")
                pts[s % n_x] = pt
                for hh in range(2):
                    nc.tensor.transpose(pt[:, hh, :], g[:, hh, :], ident[:])

            def emit_xt_cp(s):
                xt = xt_ring[s % n_x]
                pt = pts[s % n_x]
                nc.vector.tensor_copy(xt[:, :], pt.rearrange("p a b -> p (a b)"))

            # prologue
            for s in range(min(GA + 1, l_steps)):
                emit_gather(s)
            emit_xt_tr(0)
            emit_xt_cp(0)
            if l_steps > 1:
                emit_xt_tr(1)
                emit_xt_cp(1)

            # chunk pairs: phase 0 handles chunks (0,2), phase 1 chunks (1,3).
            # kj-major matmul order [0,2,1,3] matches the order h chunks are
            # produced by the previous step's tail, so the PE never waits for
            # the full h vector — only for the chunk its current MM reads.
            # PSUM region order is phase-major [c0, c2, c1, c3]: each phase
            # owns whole banks, with ONE accumulation group per bank (start
            # clears has_written bank-wide once; later first-writes to other
            # regions overwrite because their bits are cleared too).
            KJO = [0, 2, 1, 3]
            SLOT = {0: 0, 2: 1, 1: 2, 3: 3}

            def hpair(h, b):
                return h.rearrange("p (a b) x -> p a b x", a=2, b=2)[:, :, b, :]

            from contextlib import nullcontext
            loop_cm = tc.For_i(0, loop_n, 1) if loop_n else nullcontext()
            with loop_cm:
              for s in range(l_steps):
                  h_old = h_ring[s % 3]
                  h_new = h_ring[(s + 1) % 3]
                  xt = xt_ring[s % n_x]

                  # per-phase PSUM tiles (one bank each) so cross-phase ops on
                  # different banks never serialize on tile-level hazards
                  r_ab = [psra.tile([128, 2 * NB], F32, tag="ra", name=f"ra_{s}"),
                          psrb.tile([128, 2 * NB], F32, tag="rb", name=f"rb_{s}")]
                  ghn_ab = [psga.tile([128, 2 * NB], F32, tag="ga", name=f"ga_{s}"),
                            psgb.tile([128, 2 * NB], F32, tag="gb", name=f"gb_{s}")]
                  zt = {}
                  PH = {0: (0, 0), 2: (0, 1), 1: (1, 0), 3: (1, 1)}

                  def rreg(c):
                      ph, i = PH[c]
                      return r_ab[ph][:, i * NB : (i + 1) * NB]

                  def nreg(c):
                      ph, i = PH[c]
                      return ghn_ab[ph][:, i * NB : (i + 1) * NB]

                  def zreg(c):
                      ph, i = PH[c]
                      return zt[ph][:, i * NB : (i + 1) * NB]

                  r_s = gt.tile([128, 4, NB], F16, tag="r_s")
                  n_s = gt.tile([128, 4, NB], F16, tag="n_s")
                  hmn_s = gt.tile([128, 4, NB], F16, tag="hmn_s")
                  z_s = gt.tile([128, 4, NB], F16, tag="z_s")
                  t_s = gt.tile([128, 4, NB], F16, tag="t_s")

                  # copy for xt(s+1): emitted at step start so it sits in the
                  # DVE queue right after the previous step's tail, well before
                  # gin0's pt-bank handoff needs it
                  if loop_n:
                      emit_xt_cp((s + 1) % l_steps)
                  elif s >= 1 and s + 1 < l_steps:
                      emit_xt_cp(s + 1)
                  gins = {}

                  def phase_head(ph):
                      ca, cb = (0, 2) if ph == 0 else (1, 3)
                      # one group per bank: start=True only on the bank's first
                      # MM. At s==0 h is zero, the gh matmuls are skipped, and
                      # the head is the whole accumulation (stop here).
                      nc.tensor.matmul(rreg(ca), wih_t[:, ca, :], xt[:], start=True, stop=False)
                      nc.tensor.matmul(rreg(cb), wih_t[:, cb, :], xt[:], start=False, stop=(s == 0))
                      nc.tensor.matmul(nreg(ca), bhhn_t[:, ca * 128 : (ca + 1) * 128],
                                       ones_t[:], start=True, stop=False)
                      nc.tensor.matmul(nreg(cb), bhhn_t[:, cb * 128 : (cb + 1) * 128],
                                       ones_t[:], start=False, stop=(s == 0))

                  def gin_mms(ph):
                      ca, cb = (0, 2) if ph == 0 else (1, 3)
                      if ph == 0:
                          # time-share the pstr bank: ring alternates pt, gin0
                          ps_gin = pstr.tile([128, 2 * NB], F32, tag="pt", name=f"gin{ph}_{s}")
                      else:
                          ps_gin = psgin.tile([128, 2 * NB], F32, tag="gin", name=f"gin{ph}_{s}")
                      gins[ph] = ps_gin
                      nc.tensor.matmul(ps_gin[:, :NB], wih_t[:, 8 + ca, :], xt[:],
                                       start=True, stop=True)
                      nc.tensor.matmul(ps_gin[:, NB:], wih_t[:, 8 + cb, :], xt[:],
                                       start=True, stop=True)

                  def phase_mms(ph):
                      if s == 0:
                          return
                      ca, cb = (0, 2) if ph == 0 else (1, 3)
                      for kj in KJO:
                          last = kj == KJO[-1]
                          for c in (ca, cb):
                              nc.tensor.matmul(
                                  rreg(c), whh_t[:, kj, c * 128 : (c + 1) * 128],
                                  h_old[:, kj, :], start=False, stop=(last and c == cb))
                          for c in (ca, cb):
                              nc.tensor.matmul(
                                  nreg(c), whh_t[:, kj, (8 + c) * 128 : (9 + c) * 128],
                                  h_old[:, kj, :], start=False, stop=(last and c == cb))

                  def z_mms(ph):
                      ca, cb = (0, 2) if ph == 0 else (1, 3)
                      pool = psza if ph == 0 else pszb
                      zt[ph] = pool.tile([128, 2 * NB], F32, tag=f"z{ph}", name=f"z{ph}_{s}")
                      nc.tensor.matmul(zreg(ca), wih_t[:, 4 + ca, :], xt[:], start=True, stop=False)
                      nc.tensor.matmul(zreg(cb), wih_t[:, 4 + cb, :], xt[:], start=False, stop=(s == 0))
                      if s == 0:
                          return
                      for kj in KJO:
                          last = kj == KJO[-1]
                          for c in (ca, cb):
                              nc.tensor.matmul(
                                  zreg(c), whh_t[:, kj, (4 + c) * 128 : (5 + c) * 128],
                                  h_old[:, kj, :], start=False, stop=(last and c == cb))

                  sl = lambda ph: slice(2 * ph, 2 * ph + 2)
                  Sigmoid = mybir.ActivationFunctionType.Sigmoid
                  Tanh = mybir.ActivationFunctionType.Tanh

                  # ---- emission order = per-engine queue order. The ACT queue
                  # runs [r0, r1, n0, z02, n1, z13] so the cheap z sigmoids are
                  # not serialized behind the long n chain; the DVE queue runs
                  # [rm0, pre0, rm1, pre1, hmn0, t02, h02, hmn1, t13, h13] so
                  # phase 0's h tail isn't stuck behind phase 1's gate ops.
                  phase_head(0)
                  phase_mms(0)
                  phase_head(1)
                  gin_mms(0)
                  nc.scalar.activation(r_s[:, sl(0), :], r_ab[0][:], Sigmoid)
                  rm0 = gt.tile([128, 2 * NB], F32, tag="rm", name=f"rm0_{s}")
                  nc.vector.tensor_mul(rm0[:], r_s[:, sl(0), :], ghn_ab[0][:])
                  pre0 = gt.tile([128, 2 * NB], F32, tag="pre", name=f"pre0_{s}")
                  nc.vector.tensor_add(pre0[:], rm0[:], gins[0][:])
                  phase_mms(1)
                  nc.scalar.activation(r_s[:, sl(1), :], r_ab[1][:], Sigmoid)
                  gin_mms(1)
                  rm1 = gt.tile([128, 2 * NB], F32, tag="rm", name=f"rm1_{s}")
                  nc.vector.tensor_mul(rm1[:], r_s[:, sl(1), :], ghn_ab[1][:])
                  pre1 = gt.tile([128, 2 * NB], F32, tag="pre", name=f"pre1_{s}")
                  nc.vector.tensor_add(pre1[:], rm1[:], gins[1][:])
                  nc.scalar.activation(n_s[:, sl(0), :], pre0[:], Tanh)
                  z_mms(0)
                  nc.scalar.activation(z_s[:, sl(0), :], zt[0][:], Sigmoid)
                  if s == 0:
                      nc.vector.tensor_scalar_mul(hmn_s[:, sl(0), :], n_s[:, sl(0), :], -1.0)
                  else:
                      nc.vector.tensor_sub(hmn_s[:, sl(0), :], hpair(h_old, 0), n_s[:, sl(0), :])
                  nc.scalar.activation(n_s[:, sl(1), :], pre1[:], Tanh)
                  nc.vector.tensor_mul(t_s[:, sl(0), :], z_s[:, sl(0), :], hmn_s[:, sl(0), :])
                  nc.vector.tensor_add(hpair(h_new, 0), n_s[:, sl(0), :], t_s[:, sl(0), :])
                  z_mms(1)
                  nc.scalar.activation(z_s[:, sl(1), :], zt[1][:], Sigmoid)
                  if s == 0:
                      nc.vector.tensor_scalar_mul(hmn_s[:, sl(1), :], n_s[:, sl(1), :], -1.0)
                  else:
                      nc.vector.tensor_sub(hmn_s[:, sl(1), :], hpair(h_old, 1), n_s[:, sl(1), :])
                  nc.vector.tensor_mul(t_s[:, sl(1), :], z_s[:, sl(1), :], hmn_s[:, sl(1), :])
                  nc.vector.tensor_add(hpair(h_new, 1), n_s[:, sl(1), :], t_s[:, sl(1), :])

                  # ---- input prep for step s+2 at the END of the step: the pt
                  # ring then pairs gin0_{s+1} against pt_{s+2} whose freeing
                  # copy has long finished, and the transposes' WAR on pre0_s
                  # lands where the PE is anyway
                  if loop_n:
                      emit_xt_tr((s + 2) % l_steps)
                      emit_gather((s + GA + 1) % l_steps)
                  else:
                      if s + 2 < l_steps:
                          emit_xt_tr(s + 2)
                      if s + GA + 1 < l_steps:
                          emit_gather(s + GA + 1)

            nc.sync.dma_start(out=hout_d[:], in_=h_ring[l_steps % 3][:])

    nc.compile()
    return nc


def build_scores(loop_n=None):
    """128 rows of scores = hc_slice @ hr.T + row softmax, output f16.

    hr arrives in 4 column chunks so the matmuls overlap the 1MB DMA.
    exp/mul run in f16. loop_n wraps the body in For_i for HW timing.
    """
    from contextlib import nullcontext

    nc = bacc.Bacc("TRN2", target_bir_lowering=False, debug=False)
    hc_d = nc.dram_tensor("hc", [HC, 128, 128], F16, kind="ExternalInput")
    hr_d = nc.dram_tensor("hr", [HC, 128, B], F16, kind="ExternalInput")
    out_d = nc.dram_tensor("out", [128, B], F16, kind="ExternalOutput")
    NCH = 4
    CW = B // NCH  # 256

    with tile.TileContext(nc) as tc:
        with (
            tc.tile_pool(name="sb", bufs=1) as sb,
            tc.tile_pool(name="ps", bufs=1, space="PSUM") as ps,
            tc.For_i(0, loop_n, 1) if loop_n else nullcontext(),
        ):
            # single DMA engine: issue in consumption order (hc, then hr chunks)
            # so the matmuls chase the transfers
            hc_t = sb.tile([128, HC, 128], F16, tag="hc")
            nc.sync.dma_start(out=hc_t[:], in_=hc_d.rearrange("k p m -> p k m"))
            hr_t = sb.tile([128, HC, B], F16, tag="hr")
            ps_s = ps.tile([128, B], F32, tag="s")
            for j in range(NCH):
                cs = slice(j * CW, (j + 1) * CW)
                nc.sync.dma_start(out=hr_t[:, :, cs],
                                  in_=hr_d[:, :, cs].rearrange("k p n -> p k n"))
                for kj in range(HC):
                    nc.tensor.matmul(
                        ps_s[:, cs], hc_t[:, kj, :], hr_t[:, kj, cs],
                        start=(kj == 0), stop=(kj == HC - 1),
                    )
            # no max-subtraction: scores here are O(±3) and exp runs in f32,
            # so exp/sum cannot overflow; softmax result is identical
            ex = sb.tile([128, B], F16, tag="ex")
            ssum = sb.tile([128, 1], F32, tag="ssum")
            nc.scalar.activation(
                ex[:], ps_s[:], mybir.ActivationFunctionType.Exp,
                accum_out=ssum[:],
            )
            rs = sb.tile([128, 1], F32, tag="rs")
            nc.vector.reciprocal(rs[:], ssum[:])
            sm = sb.tile([128, B], F16, tag="sm")
            for hf in range(2):
                hs = slice(hf * 512, (hf + 1) * 512)
                nc.vector.tensor_scalar_mul(sm[:, hs], ex[:, hs], rs[:])
                nc.scalar.dma_start(out=out_d[:, hs], in_=sm[:, hs])

    nc.compile()
    return nc


def _prep_encoder_inputs(tok, emb16, Wih, Whh, bih, bhh):
    """Per-encoder host prep. tok [B, L] int; returns dict pieces shared by its 4 cores."""
    # wih: [E+1, 12, 128]; row E = folded bias (bih+bhh for r,z; bih for n)
    WihT = Wih.T.astype(np.float32)  # [E, 3H]
    brow = np.concatenate([
        (bih[: 2 * H] + bhh[: 2 * H]),
        bih[2 * H :],
    ]).astype(np.float32)  # [3H]
    wih = np.concatenate([WihT, brow[None, :]], axis=0)  # [E+1, 3H]
    wih = np.ascontiguousarray(
        wih.reshape(E + 1, 12, 128)
    ).astype(np.float16)
    # whh: [128, HC, 3H]: whh[p, kj, m] = Whh.T[kj*128+p, m] = Whh[m, kj*128+p]
    whh = np.ascontiguousarray(
        Whh.T.astype(np.float32).reshape(HC, 128, 3 * H).transpose(1, 0, 2)
    ).astype(np.float16)
    bhhn = bhh[2 * H :].astype(np.float16)[None, :]  # [1, H]
    return wih, whh, bhhn


def _prep_idx(tok_shard):
    """tok_shard [NB, L] -> idx [128, 2L] int32: idx[p, 2s+h] = tok[h*128+p, L-1-s]."""
    t = tok_shard.reshape(2, 128, L)          # [h, p, l]
    rev = t[:, :, ::-1]                        # l -> step s
    idx = rev.transpose(1, 2, 0).reshape(128, L * 2)  # [p, (s, h)]
    return np.ascontiguousarray(idx).astype(np.int32)


_CACHE = {}
TRACE = False
LAST_EXEC_NS = {}


def kernel(contexts, replies, ctx_emb, ctx_Wih, ctx_Whh, ctx_bih, ctx_bhh,
           rep_emb, rep_Wih, rep_Whh, rep_bih, rep_bhh):
    contexts = np.asarray(contexts).astype(np.int32)
    replies = np.asarray(replies).astype(np.int32)
    as32 = lambda a: np.asarray(a, dtype=np.float32)
    ctx_emb16 = as32(ctx_emb).astype(np.float16)
    rep_emb16 = as32(rep_emb).astype(np.float16)

    if "enc" not in _CACHE:
        _CACHE["enc"] = build_encoder()
    if "sco" not in _CACHE:
        _CACHE["sco"] = build_scores()
    enc = _CACHE["enc"]
    sco = _CACHE["sco"]

    cw = _prep_encoder_inputs(contexts, ctx_emb16, as32(ctx_Wih), as32(ctx_Whh),
                              as32(ctx_bih), as32(ctx_bhh))
    rw = _prep_encoder_inputs(replies, rep_emb16, as32(rep_Wih), as32(rep_Whh),
                              as32(rep_bih), as32(rep_bhh))

    in_maps = []
    for c in range(8):
        if c < 4:
            tok, emb16, (wih, whh, bhhn) = contexts, ctx_emb16, cw
            sh = c
        else:
            tok, emb16, (wih, whh, bhhn) = replies, rep_emb16, rw
            sh = c - 4
        in_maps.append({
            "emb": emb16,
            "idx": _prep_idx(tok[sh * NB : (sh + 1) * NB]),
            "wih": wih,
            "whh": whh,
            "bhhn": bhhn,
        })

    res = run_bass_kernel_spmd(enc, in_maps, core_ids=list(range(8)), trace=TRACE)
    if TRACE:
        LAST_EXEC_NS["enc"] = res.exec_time_ns
    houts = [r["hout"] for r in res.results]  # each [128, HC, NB] fp16

    # assemble hcT_all / hrT_all: [HC, 128, B] fp16 (feature-chunked, batch on free)
    hcT = np.concatenate([houts[c].transpose(1, 0, 2) for c in range(4)], axis=2)
    hrT = np.concatenate([houts[c].transpose(1, 0, 2) for c in range(4, 8)], axis=2)

    in_maps2 = []
    for c in range(8):
        in_maps2.append({
            "hc": np.ascontiguousarray(hcT[:, :, c * 128 : (c + 1) * 128]),
            "hr": np.ascontiguousarray(hrT),
        })
    res2 = run_bass_kernel_spmd(sco, in_maps2, core_ids=list(range(8)), trace=TRACE)
    if TRACE:
        LAST_EXEC_NS["sco"] = res2.exec_time_ns
    out = np.concatenate([r["out"] for r in res2.results], axis=0)
    return out.astype(np.float32)



# revision 2
# speedup vs baseline: 1.7700x; 1.7700x over previous
"""DSSM (dual GRU encoder + BxB softmax similarity) on 8 Trainium2 NeuronCores.

Strategy:
  - Cores 0-3 run the context encoder on batch rows [256c, 256(c+1));
    cores 4-7 run the reply encoder on rows [256(c-4), 256(c-3)).
    One SPMD program; per-core in_maps carry the right table/weights/indices.
  - Encoder: backward GRU over L=128 steps, hidden state kept transposed
    (features on partitions): h as [128 part, 4 chunk, 256 batch] fp16.
    Per step: 66 fp16 matmuls (gi K=101 incl. folded biases via ones column,
    gh K=128, bhh_n via K=1 ones outer product) accumulate gate
    pre-activations in PSUM; ACT does sigmoid/tanh; DVE combines.
    Embedding rows arrive via per-step indirect-DMA gathers (+ PE transpose).
  - Phase 2 (second small kernel): each core computes a 128-row slice of
    scores = hc @ hr.T (fp16 matmuls, fp32 accum) + row softmax.

All tensor layout prep (transposes, bias folding, sharding, time reversal)
is host-side numpy; the FLOP-carrying work runs on device.
"""

import numpy as np

import concourse.bass as bass
import concourse.mybir as mybir
import concourse.tile as tile
from concourse import bacc
from concourse.bass_utils import run_bass_kernel_spmd
from concourse.masks import make_identity

F16 = mybir.dt.float16
F32 = mybir.dt.float32
I32 = mybir.dt.int32
F8 = mybir.dt.float8e4
NP8 = mybir.dt.np(F8)
DR = mybir.MatmulPerfMode.DoubleRow

V, E, H, B, L = 50000, 100, 512, 1024, 128
NB = 256          # batch rows per core
HC = H // 128     # 4 h chunks
GA = 6            # gather lookahead (steps)
XA = 2            # xt lookahead


def build_encoder(l_steps=L, loop_n=None):
    """loop_n: benchmark-only mode — wraps the step loop in a For_i that
    re-runs the whole sequence loop_n times (data goes stale; timing only)."""
    nc = bacc.Bacc("TRN2", target_bir_lowering=False, debug=False)
    emb_d = nc.dram_tensor("emb", [V, E], F16, kind="ExternalInput")
    idx_d = nc.dram_tensor("idx", [128, 2 * l_steps], I32, kind="ExternalInput")
    wih_d = nc.dram_tensor("wih", [E + 1, 12, 128], F16, kind="ExternalInput")
    whh_d = nc.dram_tensor("whh", [128, HC, 3 * H], F16, kind="ExternalInput")
    bhhn_d = nc.dram_tensor("bhhn", [1, H], F16, kind="ExternalInput")
    hout_d = nc.dram_tensor("hout", [128, HC, NB], F16, kind="ExternalOutput")

    with tile.TileContext(nc) as tc:
        with (
            tc.tile_pool(name="wt", bufs=1) as wt,
            tc.tile_pool(name="grng", bufs=GA + 2) as grng,
            tc.tile_pool(name="xrng", bufs=XA + 2) as xrng,
            tc.tile_pool(name="hrng", bufs=3) as hrng,
            tc.tile_pool(name="gt", bufs=4) as gt,
            # one PSUM pool per bank; pstr also hosts gin0 (time-shared slot)
            tc.tile_pool(name="psra", bufs=1, space="PSUM") as psra,
            tc.tile_pool(name="psrb", bufs=1, space="PSUM") as psrb,
            tc.tile_pool(name="psza", bufs=1, space="PSUM") as psza,
            tc.tile_pool(name="pszb", bufs=1, space="PSUM") as pszb,
            tc.tile_pool(name="psga", bufs=1, space="PSUM") as psga,
            tc.tile_pool(name="psgb", bufs=1, space="PSUM") as psgb,
            tc.tile_pool(name="psgin", bufs=1, space="PSUM") as psgin,
            tc.tile_pool(name="pstr", bufs=1, space="PSUM") as pstr,
        ):
            # --- constants / weights (idx first: gathers need it; whh last:
            # step 0 skips its gh matmuls since h0 == 0, so whh is only
            # needed from step 1) ---
            idx_t = wt.tile([128, 2 * l_steps], I32, tag="idx")
            nc.sync.dma_start(out=idx_t[:], in_=idx_d[:])
            wih_t = wt.tile([E + 1, 12, 128], F16, tag="wih")
            nc.sync.dma_start(out=wih_t[:], in_=wih_d[:])
            bhhn_t = wt.tile([1, H], F16, tag="bhhn")
            nc.sync.dma_start(out=bhhn_t[:], in_=bhhn_d[:])
            # whh split r/n/z in consumption order: step 1's r matmuls can
            # start ~3us before the z columns finish loading
            whh_t = wt.tile([128, HC, 3 * H], F16, tag="whh")
            nc.sync.dma_start(out=whh_t[:, :, 0 : H], in_=whh_d[:, :, 0 : H])
            nc.sync.dma_start(out=whh_t[:, :, 2 * H : 3 * H], in_=whh_d[:, :, 2 * H : 3 * H])
            nc.sync.dma_start(out=whh_t[:, :, H : 2 * H], in_=whh_d[:, :, H : 2 * H])
            ident = wt.tile([128, 128], F16, tag="ident")
            make_identity(nc, ident[:])
            ones_t = wt.tile([1, NB], F16, tag="ones")
            nc.vector.memset(ones_t[:], 1.0)

            # --- rings ---
            n_g = GA + 2
            g_ring = []
            for i in range(n_g):
                g = grng.tile([128, 2, E + 1], F16, tag="g", name=f"g{i}")
                nc.vector.memset(g[:, :, E : E + 1], 1.0)
                g_ring.append(g)
            n_x = XA + 2
            xt_ring = [xrng.tile([E + 1, NB], F16, tag="xt", name=f"xt{i}") for i in range(n_x)]
            # no h memsets: step 0 skips all gh matmuls and computes its hmn
            # as -n directly, so no slot is ever read before it is written
            h_ring = [hrng.tile([128, HC, NB], F16, tag="h", name=f"h{i}")
                      for i in range(3)]

            def emit_gather(s):
                g = g_ring[s % n_g]
                for hh in range(2):
                    nc.gpsimd.indirect_dma_start(
                        out=g[:, hh, :E],
                        out_offset=None,
                        in_=emb_d[:],
                        in_offset=bass.IndirectOffsetOnAxis(
                            ap=idx_t[:, 2 * s + hh : 2 * s + hh + 1], axis=0
                        ),
                    )

            pts = {}

            def emit_xt_tr(s):
                g = g_ring[s % n_g]
                pt = pstr.tile([E + 1, 2, 128], F16, tag="pt", name=f"pt{s}")
                pts[s % n_x] = pt
                for hh in range(2):
                    nc.tensor.transpose(pt[:, hh, :], g[:, hh, :], ident[:])

            def emit_xt_cp(s):
                xt = xt_ring[s % n_x]
                pt = pts[s % n_x]
                nc.vector.tensor_copy(xt[:, :], pt.rearrange("p a b -> p (a b)"))

            # prologue
            for s in range(min(GA + 1, l_steps)):
                emit_gather(s)
            emit_xt_tr(0)
            emit_xt_cp(0)
            if l_steps > 1:
                emit_xt_tr(1)
                emit_xt_cp(1)

            # chunk pairs: phase 0 handles chunks (0,2), phase 1 chunks (1,3).
            # kj-major matmul order [0,2,1,3] matches the order h chunks are
            # produced by the previous step's tail, so the PE never waits for
            # the full h vector — only for the chunk its current MM reads.
            # PSUM region order is phase-major [c0, c2, c1, c3]: each phase
            # owns whole banks, with ONE accumulation group per bank (start
            # clears has_written bank-wide once; later first-writes to other
            # regions overwrite because their bits are cleared too).
            KJO = [0, 2, 1, 3]
            SLOT = {0: 0, 2: 1, 1: 2, 3: 3}

            def hpair(h, b):
                return h.rearrange("p (a b) x -> p a b x", a=2, b=2)[:, :, b, :]

            from contextlib import nullcontext
            loop_cm = tc.For_i(0, loop_n, 1) if loop_n else nullcontext()
            with loop_cm:
              for s in range(l_steps):
                  h_old = h_ring[s % 3]
                  h_new = h_ring[(s + 1) % 3]
                  xt = xt_ring[s % n_x]

                  # per-phase PSUM tiles (one bank each) so cross-phase ops on
                  # different banks never serialize on tile-level hazards
                  r_ab = [psra.tile([128, 2 * NB], F32, tag="ra", name=f"ra_{s}"),
                          psrb.tile([128, 2 * NB], F32, tag="rb", name=f"rb_{s}")]
                  ghn_ab = [psga.tile([128, 2 * NB], F32, tag="ga", name=f"ga_{s}"),
                            psgb.tile([128, 2 * NB], F32, tag="gb", name=f"gb_{s}")]
                  zt = {}
                  PH = {0: (0, 0), 2: (0, 1), 1: (1, 0), 3: (1, 1)}

                  def rreg(c):
                      ph, i = PH[c]
                      return r_ab[ph][:, i * NB : (i + 1) * NB]

                  def nreg(c):
                      ph, i = PH[c]
                      return ghn_ab[ph][:, i * NB : (i + 1) * NB]

                  def zreg(c):
                      ph, i = PH[c]
                      return zt[ph][:, i * NB : (i + 1) * NB]

                  r_s = gt.tile([128, 4, NB], F16, tag="r_s")
                  n_s = gt.tile([128, 4, NB], F16, tag="n_s")
                  hmn_s = gt.tile([128, 4, NB], F16, tag="hmn_s")
                  z_s = gt.tile([128, 4, NB], F16, tag="z_s")
                  t_s = gt.tile([128, 4, NB], F16, tag="t_s")

                  # copy for xt(s+1): emitted at step start so it sits in the
                  # DVE queue right after the previous step's tail, well before
                  # gin0's pt-bank handoff needs it
                  if loop_n:
                      emit_xt_cp((s + 1) % l_steps)
                  elif s >= 1 and s + 1 < l_steps:
                      emit_xt_cp(s + 1)
                  gins = {}

                  def phase_head(ph):
                      ca, cb = (0, 2) if ph == 0 else (1, 3)
                      # one group per bank: start=True only on the bank's first
                      # MM. At s==0 h is zero, the gh matmuls are skipped, and
                      # the head is the whole accumulation (stop here).
                      nc.tensor.matmul(rreg(ca), wih_t[:, ca, :], xt[:], start=True, stop=False)
                      nc.tensor.matmul(rreg(cb), wih_t[:, cb, :], xt[:], start=False, stop=(s == 0))
                      nc.tensor.matmul(nreg(ca), bhhn_t[:, ca * 128 : (ca + 1) * 128],
                                       ones_t[:], start=True, stop=False)
                      nc.tensor.matmul(nreg(cb), bhhn_t[:, cb * 128 : (cb + 1) * 128],
                                       ones_t[:], start=False, stop=(s == 0))

                  def gin_mms(ph):
                      ca, cb = (0, 2) if ph == 0 else (1, 3)
                      if ph == 0:
                          # time-share the pstr bank: ring alternates pt, gin0
                          ps_gin = pstr.tile([128, 2 * NB], F32, tag="pt", name=f"gin{ph}_{s}")
                      else:
                          ps_gin = psgin.tile([128, 2 * NB], F32, tag="gin", name=f"gin{ph}_{s}")
                      gins[ph] = ps_gin
                      nc.tensor.matmul(ps_gin[:, :NB], wih_t[:, 8 + ca, :], xt[:],
                                       start=True, stop=True)
                      nc.tensor.matmul(ps_gin[:, NB:], wih_t[:, 8 + cb, :], xt[:],
                                       start=True, stop=True)

                  def phase_mms(ph):
                      if s == 0:
                          return
                      ca, cb = (0, 2) if ph == 0 else (1, 3)
                      for kj in KJO:
                          last = kj == KJO[-1]
                          for c in (ca, cb):
                              nc.tensor.matmul(
                                  rreg(c), whh_t[:, kj, c * 128 : (c + 1) * 128],
                                  h_old[:, kj, :], start=False, stop=(last and c == cb))
                          for c in (ca, cb):
                              nc.tensor.matmul(
                                  nreg(c), whh_t[:, kj, (8 + c) * 128 : (9 + c) * 128],
                                  h_old[:, kj, :], start=False, stop=(last and c == cb))

                  def z_mms(ph):
                      ca, cb = (0, 2) if ph == 0 else (1, 3)
                      pool = psza if ph == 0 else pszb
                      zt[ph] = pool.tile([128, 2 * NB], F32, tag=f"z{ph}", name=f"z{ph}_{s}")
                      nc.tensor.matmul(zreg(ca), wih_t[:, 4 + ca, :], xt[:], start=True, stop=False)
                      nc.tensor.matmul(zreg(cb), wih_t[:, 4 + cb, :], xt[:], start=False, stop=(s == 0))
                      if s == 0:
                          return
                      for kj in KJO:
                          last = kj == KJO[-1]
                          for c in (ca, cb):
                              nc.tensor.matmul(
                                  zreg(c), whh_t[:, kj, (4 + c) * 128 : (5 + c) * 128],
                                  h_old[:, kj, :], start=False, stop=(last and c == cb))

                  sl = lambda ph: slice(2 * ph, 2 * ph + 2)
                  Sigmoid = mybir.ActivationFunctionType.Sigmoid
                  Tanh = mybir.ActivationFunctionType.Tanh

                  # ---- emission order = per-engine queue order. The ACT queue
                  # runs [r0, r1, n0, z02, n1, z13] so the cheap z sigmoids are
                  # not serialized behind the long n chain; the DVE queue runs
                  # [rm0, pre0, rm1, pre1, hmn0, t02, h02, hmn1, t13, h13] so
                  # phase 0's h tail isn't stuck behind phase 1's gate ops.
                  phase_head(0)
                  phase_mms(0)
                  phase_head(1)
                  gin_mms(0)
                  nc.scalar.activation(r_s[:, sl(0), :], r_ab[0][:], Sigmoid)
                  rm0 = gt.tile([128, 2 * NB], F32, tag="rm", name=f"rm0_{s}")
                  nc.vector.tensor_mul(rm0[:], r_s[:, sl(0), :], ghn_ab[0][:])
                  pre0 = gt.tile([128, 2 * NB], F32, tag="pre", name=f"pre0_{s}")
                  nc.vector.tensor_add(pre0[:], rm0[:], gins[0][:])
                  phase_mms(1)
                  nc.scalar.activation(r_s[:, sl(1), :], r_ab[1][:], Sigmoid)
                  gin_mms(1)
                  rm1 = gt.tile([128, 2 * NB], F32, tag="rm", name=f"rm1_{s}")
                  nc.vector.tensor_mul(rm1[:], r_s[:, sl(1), :], ghn_ab[1][:])
                  pre1 = gt.tile([128, 2 * NB], F32, tag="pre", name=f"pre1_{s}")
                  nc.vector.tensor_add(pre1[:], rm1[:], gins[1][:])
                  nc.scalar.activation(n_s[:, sl(0), :], pre0[:], Tanh)
                  z_mms(0)
                  nc.scalar.activation(z_s[:, sl(0), :], zt[0][:], Sigmoid)
                  if s == 0:
                      nc.vector.tensor_scalar_mul(hmn_s[:, sl(0), :], n_s[:, sl(0), :], -1.0)
                  else:
                      nc.vector.tensor_sub(hmn_s[:, sl(0), :], hpair(h_old, 0), n_s[:, sl(0), :])
                  nc.scalar.activation(n_s[:, sl(1), :], pre1[:], Tanh)
                  nc.vector.tensor_mul(t_s[:, sl(0), :], z_s[:, sl(0), :], hmn_s[:, sl(0), :])
                  nc.vector.tensor_add(hpair(h_new, 0), n_s[:, sl(0), :], t_s[:, sl(0), :])
                  z_mms(1)
                  nc.scalar.activation(z_s[:, sl(1), :], zt[1][:], Sigmoid)
                  if s == 0:
                      nc.vector.tensor_scalar_mul(hmn_s[:, sl(1), :], n_s[:, sl(1), :], -1.0)
                  else:
                      nc.vector.tensor_sub(hmn_s[:, sl(1), :], hpair(h_old, 1), n_s[:, sl(1), :])
                  nc.vector.tensor_mul(t_s[:, sl(1), :], z_s[:, sl(1), :], hmn_s[:, sl(1), :])
                  nc.vector.tensor_add(hpair(h_new, 1), n_s[:, sl(1), :], t_s[:, sl(1), :])

                  # ---- input prep for step s+2 at the END of the step: the pt
                  # ring then pairs gin0_{s+1} against pt_{s+2} whose freeing
                  # copy has long finished, and the transposes' WAR on pre0_s
                  # lands where the PE is anyway
                  if loop_n:
                      emit_xt_tr((s + 2) % l_steps)
                      emit_gather((s + GA + 1) % l_steps)
                  else:
                      if s + 2 < l_steps:
                          emit_xt_tr(s + 2)
                      if s + GA + 1 < l_steps:
                          emit_gather(s + GA + 1)

            nc.sync.dma_start(out=hout_d[:], in_=h_ring[l_steps % 3][:])

    nc.compile()
    return nc


def build_scores(loop_n=None):
    """128 rows of scores = hc8 @ hr8.T (fp8 DoubleRow) + row softmax, out f16.

    hc8 [p, kt, pl, m] = hc[128*core + m, (2kt+pl)*128 + p]; hr8 likewise over
    all B columns. 8 DR matmuls (K=256 each) accumulate the K=512 contraction.
    """
    from contextlib import nullcontext

    nc = bacc.Bacc("TRN2", target_bir_lowering=False, debug=False)
    hc_d = nc.dram_tensor("hc", [128, 2, 2, 128], F8, kind="ExternalInput")
    hr_d = nc.dram_tensor("hr", [128, 2, 2, B], F8, kind="ExternalInput")
    out_d = nc.dram_tensor("out", [128, B], F16, kind="ExternalOutput")
    NCH = 4
    CW = B // NCH  # 256

    with tile.TileContext(nc) as tc:
        with (
            tc.tile_pool(name="sb", bufs=1) as sb,
            tc.tile_pool(name="ps", bufs=1, space="PSUM") as ps,
            tc.For_i(0, loop_n, 1) if loop_n else nullcontext(),
        ):
            hc_t = sb.tile([128, 2, 2, 128], F8, tag="hc")
            nc.sync.dma_start(out=hc_t[:], in_=hc_d[:])
            hr_t = sb.tile([128, 2, 2, B], F8, tag="hr")
            ps_s = ps.tile([128, B], F32, tag="s")
            for j in range(NCH):
                cs = slice(j * CW, (j + 1) * CW)
                nc.sync.dma_start(out=hr_t[:, :, :, cs], in_=hr_d[:, :, :, cs])
                for kt in range(2):
                    nc.tensor.matmul(
                        ps_s[:, cs], hc_t[:, kt, :, :], hr_t[:, kt, :, cs],
                        start=(kt == 0), stop=(kt == 1), perf_mode=DR)
            ex = sb.tile([128, B], F16, tag="ex")
            ssum = sb.tile([128, 1], F32, tag="ssum")
            nc.scalar.activation(
                ex[:], ps_s[:], mybir.ActivationFunctionType.Exp,
                accum_out=ssum[:])
            rs = sb.tile([128, 1], F32, tag="rs")
            nc.vector.reciprocal(rs[:], ssum[:])
            sm = sb.tile([128, B], F16, tag="sm")
            for hf in range(2):
                hs = slice(hf * 512, (hf + 1) * 512)
                nc.vector.tensor_scalar_mul(sm[:, hs], ex[:, hs], rs[:])
                nc.scalar.dma_start(out=out_d[:, hs], in_=sm[:, hs])

    nc.compile()
    return nc


def _prep_encoder_inputs(tok, emb16, Wih, Whh, bih, bhh):
    """Per-encoder host prep. tok [B, L] int; returns dict pieces shared by its 4 cores."""
    # wih: [E+1, 12, 128]; row E = folded bias (bih+bhh for r,z; bih for n)
    WihT = Wih.T.astype(np.float32)  # [E, 3H]
    brow = np.concatenate([
        (bih[: 2 * H] + bhh[: 2 * H]),
        bih[2 * H :],
    ]).astype(np.float32)  # [3H]
    wih = np.concatenate([WihT, brow[None, :]], axis=0)  # [E+1, 3H]
    wih = np.ascontiguousarray(
        wih.reshape(E + 1, 12, 128)
    ).astype(np.float16)
    # whh: [128, HC, 3H]: whh[p, kj, m] = Whh.T[kj*128+p, m] = Whh[m, kj*128+p]
    whh = np.ascontiguousarray(
        Whh.T.astype(np.float32).reshape(HC, 128, 3 * H).transpose(1, 0, 2)
    ).astype(np.float16)
    bhhn = bhh[2 * H :].astype(np.float16)[None, :]  # [1, H]
    return wih, whh, bhhn


def _prep_idx(tok_shard):
    """tok_shard [NB, L] -> idx [128, 2L] int32: idx[p, 2s+h] = tok[h*128+p, L-1-s]."""
    t = tok_shard.reshape(2, 128, L)          # [h, p, l]
    rev = t[:, :, ::-1]                        # l -> step s
    idx = rev.transpose(1, 2, 0).reshape(128, L * 2)  # [p, (s, h)]
    return np.ascontiguousarray(idx).astype(np.int32)


_CACHE = {}
TRACE = False
LAST_EXEC_NS = {}


def kernel(contexts, replies, ctx_emb, ctx_Wih, ctx_Whh, ctx_bih, ctx_bhh,
           rep_emb, rep_Wih, rep_Whh, rep_bih, rep_bhh):
    contexts = np.asarray(contexts).astype(np.int32)
    replies = np.asarray(replies).astype(np.int32)
    as32 = lambda a: np.asarray(a, dtype=np.float32)
    ctx_emb16 = as32(ctx_emb).astype(np.float16)
    rep_emb16 = as32(rep_emb).astype(np.float16)

    if "enc" not in _CACHE:
        _CACHE["enc"] = build_encoder()
    if "sco" not in _CACHE:
        _CACHE["sco"] = build_scores()
    enc = _CACHE["enc"]
    sco = _CACHE["sco"]

    cw = _prep_encoder_inputs(contexts, ctx_emb16, as32(ctx_Wih), as32(ctx_Whh),
                              as32(ctx_bih), as32(ctx_bhh))
    rw = _prep_encoder_inputs(replies, rep_emb16, as32(rep_Wih), as32(rep_Whh),
                              as32(rep_bih), as32(rep_bhh))

    in_maps = []
    for c in range(8):
        if c < 4:
            tok, emb16, (wih, whh, bhhn) = contexts, ctx_emb16, cw
            sh = c
        else:
            tok, emb16, (wih, whh, bhhn) = replies, rep_emb16, rw
            sh = c - 4
        in_maps.append({
            "emb": emb16,
            "idx": _prep_idx(tok[sh * NB : (sh + 1) * NB]),
            "wih": wih,
            "whh": whh,
            "bhhn": bhhn,
        })

    res = run_bass_kernel_spmd(enc, in_maps, core_ids=list(range(8)), trace=TRACE)
    if TRACE:
        LAST_EXEC_NS["enc"] = res.exec_time_ns
    houts = [r["hout"] for r in res.results]  # each [128, HC, NB] fp16

    # assemble fp8 score inputs: [ch, p, B] -> [p, kt, pl, B] with ch = 2kt+pl
    hcT = np.concatenate([houts[c].transpose(1, 0, 2) for c in range(4)], axis=2)
    hrT = np.concatenate([houts[c].transpose(1, 0, 2) for c in range(4, 8)], axis=2)
    to8 = lambda a: np.clip(a.astype(np.float32), -240, 240).astype(NP8)
    hc_all = np.ascontiguousarray(
        to8(hcT).reshape(2, 2, 128, B).transpose(2, 0, 1, 3))
    hr_all = np.ascontiguousarray(
        to8(hrT).reshape(2, 2, 128, B).transpose(2, 0, 1, 3))

    in_maps2 = []
    for c in range(8):
        in_maps2.append({
            "hc": np.ascontiguousarray(hc_all[:, :, :, c * 128 : (c + 1) * 128]),
            "hr": hr_all,
        })
    res2 = run_bass_kernel_spmd(sco, in_maps2, core_ids=list(range(8)), trace=TRACE)
    if TRACE:
        LAST_EXEC_NS["sco"] = res2.exec_time_ns
    out = np.concatenate([r["out"] for r in res2.results], axis=0)
    return out.astype(np.float32)



# revision 3
# speedup vs baseline: 2.1238x; 1.1999x over previous
"""DSSM (dual GRU encoder + BxB softmax similarity) on 8 Trainium2 NeuronCores.

Strategy:
  - Cores 0-3 run the context encoder on batch rows [256c, 256(c+1));
    cores 4-7 run the reply encoder on rows [256(c-4), 256(c-3)).
    One SPMD program; per-core in_maps carry the right table/weights/indices.
  - Encoder: backward GRU over L=128 steps, hidden state kept transposed
    (features on partitions): h as [128 part, 4 chunk, 256 batch] fp16.
    Per step: 66 fp16 matmuls (gi K=101 incl. folded biases via ones column,
    gh K=128, bhh_n via K=1 ones outer product) accumulate gate
    pre-activations in PSUM; ACT does sigmoid/tanh; DVE combines.
    Embedding rows arrive via per-step indirect-DMA gathers (+ PE transpose).
  - Phase 2 (second small kernel): each core computes a 128-row slice of
    scores = hc @ hr.T (fp16 matmuls, fp32 accum) + row softmax.

All tensor layout prep (transposes, bias folding, sharding, time reversal)
is host-side numpy; the FLOP-carrying work runs on device.
"""

import numpy as np

import concourse.bass as bass
import concourse.mybir as mybir
import concourse.tile as tile
from concourse import bacc
from concourse.bass_utils import run_bass_kernel_spmd
from concourse.masks import make_identity

F16 = mybir.dt.float16
F32 = mybir.dt.float32
I32 = mybir.dt.int32
F8 = mybir.dt.float8e4
NP8 = mybir.dt.np(F8)
DR = mybir.MatmulPerfMode.DoubleRow
Copy = mybir.ActivationFunctionType.Copy
S = 64.0
INV = float(1.0 / S)

V, E, H, B, L = 50000, 100, 512, 1024, 128
NB = 256          # batch rows per core
HC = H // 128     # 4 h chunks
GA = 6            # gather lookahead (steps)
XA = 2            # xt lookahead


def build_encoder(l_steps=L, loop_n=None):
    """loop_n: benchmark-only mode — wraps the step loop in a For_i that
    re-runs the whole sequence loop_n times (data goes stale; timing only)."""
    nc = bacc.Bacc("TRN2", target_bir_lowering=False, debug=False)
    emb_d = nc.dram_tensor("emb", [V, E], F16, kind="ExternalInput")
    idx_d = nc.dram_tensor("idx", [128, 2 * l_steps], I32, kind="ExternalInput")
    wih_d = nc.dram_tensor("wih", [E + 1, 12, 128], F16, kind="ExternalInput")
    whh_d = nc.dram_tensor("whh", [128, 2, 2, 3 * H], F8, kind="ExternalInput")
    bhhn_d = nc.dram_tensor("bhhn", [1, H], F16, kind="ExternalInput")
    hout_d = nc.dram_tensor("hout", [128, HC, NB], F16, kind="ExternalOutput")

    with tile.TileContext(nc) as tc:
        with (
            tc.tile_pool(name="wt", bufs=1) as wt,
            tc.tile_pool(name="grng", bufs=GA + 2) as grng,
            tc.tile_pool(name="xrng", bufs=XA + 2) as xrng,
            tc.tile_pool(name="hrng", bufs=3) as hrng,
            tc.tile_pool(name="gt", bufs=4) as gt,
            # one PSUM pool per bank; pstr also hosts gin0 (time-shared slot)
            tc.tile_pool(name="psra", bufs=1, space="PSUM") as psra,
            tc.tile_pool(name="psrb", bufs=1, space="PSUM") as psrb,
            tc.tile_pool(name="psza", bufs=1, space="PSUM") as psza,
            tc.tile_pool(name="pszb", bufs=1, space="PSUM") as pszb,
            tc.tile_pool(name="psga", bufs=1, space="PSUM") as psga,
            tc.tile_pool(name="psgb", bufs=1, space="PSUM") as psgb,
            tc.tile_pool(name="psgin", bufs=1, space="PSUM") as psgin,
            tc.tile_pool(name="pstr", bufs=1, space="PSUM") as pstr,
        ):
            # --- constants / weights (idx first: gathers need it; whh last:
            # step 0 skips its gh matmuls since h0 == 0, so whh is only
            # needed from step 1) ---
            idx_t = wt.tile([128, 2 * l_steps], I32, tag="idx")
            nc.sync.dma_start(out=idx_t[:], in_=idx_d[:])
            wih_t = wt.tile([E + 1, 12, 128], F16, tag="wih")
            nc.sync.dma_start(out=wih_t[:], in_=wih_d[:])
            bhhn_t = wt.tile([1, H], F16, tag="bhhn")
            nc.sync.dma_start(out=bhhn_t[:], in_=bhhn_d[:])
            # whh split r/n/z in consumption order: step 1's r matmuls can
            # start ~3us before the z columns finish loading
            whh_t = wt.tile([128, 2, 2, 3 * H], F8, tag="whh")
            nc.sync.dma_start(out=whh_t[:, :, :, 0 : H], in_=whh_d[:, :, :, 0 : H])
            nc.sync.dma_start(out=whh_t[:, :, :, 2 * H : 3 * H], in_=whh_d[:, :, :, 2 * H : 3 * H])
            nc.sync.dma_start(out=whh_t[:, :, :, H : 2 * H], in_=whh_d[:, :, :, H : 2 * H])
            ident = wt.tile([128, 128], F16, tag="ident")
            make_identity(nc, ident[:])
            ones_t = wt.tile([1, NB], F16, tag="ones")
            nc.vector.memset(ones_t[:], 1.0)

            # --- rings ---
            n_g = GA + 2
            g_ring = []
            for i in range(n_g):
                g = grng.tile([128, 2, E + 1], F16, tag="g", name=f"g{i}")
                nc.vector.memset(g[:, :, E : E + 1], 1.0)
                g_ring.append(g)
            n_x = XA + 2
            xt_ring = [xrng.tile([E + 1, NB], F16, tag="xt", name=f"xt{i}") for i in range(n_x)]
            # no h memsets: step 0 skips all gh matmuls and computes its hmn
            # as -n directly, so no slot is ever read before it is written
            h_ring = [hrng.tile([128, HC, NB], F16, tag="h", name=f"h{i}")
                      for i in range(3)]
            h8_ring = [hrng.tile([128, HC, NB], F8, tag="h8", name=f"h8{i}")
                       for i in range(3)]

            def emit_gather(s):
                g = g_ring[s % n_g]
                for hh in range(2):
                    nc.gpsimd.indirect_dma_start(
                        out=g[:, hh, :E],
                        out_offset=None,
                        in_=emb_d[:],
                        in_offset=bass.IndirectOffsetOnAxis(
                            ap=idx_t[:, 2 * s + hh : 2 * s + hh + 1], axis=0
                        ),
                    )

            pts = {}

            def emit_xt_tr(s):
                g = g_ring[s % n_g]
                pt = pstr.tile([E + 1, 2, 128], F16, tag="pt", name=f"pt{s}")
                pts[s % n_x] = pt
                for hh in range(2):
                    nc.tensor.transpose(pt[:, hh, :], g[:, hh, :], ident[:])

            def emit_xt_cp(s):
                xt = xt_ring[s % n_x]
                pt = pts[s % n_x]
                nc.vector.tensor_copy(xt[:, :], pt.rearrange("p a b -> p (a b)"))

            # prologue
            for s in range(min(GA + 1, l_steps)):
                emit_gather(s)
            emit_xt_tr(0)
            emit_xt_cp(0)
            if l_steps > 1:
                emit_xt_tr(1)
                emit_xt_cp(1)

            # chunk pairs: phase 0 handles chunks (0,2), phase 1 chunks (1,3).
            # kj-major matmul order [0,2,1,3] matches the order h chunks are
            # produced by the previous step's tail, so the PE never waits for
            # the full h vector — only for the chunk its current MM reads.
            # PSUM region order is phase-major [c0, c2, c1, c3]: each phase
            # owns whole banks, with ONE accumulation group per bank (start
            # clears has_written bank-wide once; later first-writes to other
            # regions overwrite because their bits are cleared too).
            def hpair(h, b):
                return h[:, 2 * b : 2 * b + 2, :]

            from contextlib import nullcontext
            loop_cm = tc.For_i(0, loop_n, 1) if loop_n else nullcontext()
            with loop_cm:
              for s in range(l_steps):
                  h_old = h_ring[s % 3]
                  h_new = h_ring[(s + 1) % 3]
                  h8_old = h8_ring[s % 3]
                  h8_new = h8_ring[(s + 1) % 3]
                  xt = xt_ring[s % n_x]

                  # per-phase PSUM tiles (one bank each) so cross-phase ops on
                  # different banks never serialize on tile-level hazards
                  r_ab = [psra.tile([128, 2 * NB], F32, tag="ra", name=f"ra_{s}"),
                          psrb.tile([128, 2 * NB], F32, tag="rb", name=f"rb_{s}")]
                  ghn_ab = [psga.tile([128, 2 * NB], F32, tag="ga", name=f"ga_{s}"),
                            psgb.tile([128, 2 * NB], F32, tag="gb", name=f"gb_{s}")]
                  zt = {}
                  PH = {0: (0, 0), 1: (0, 1), 2: (1, 0), 3: (1, 1)}

                  def rreg(c):
                      ph, i = PH[c]
                      return r_ab[ph][:, i * NB : (i + 1) * NB]

                  def nreg(c):
                      ph, i = PH[c]
                      return ghn_ab[ph][:, i * NB : (i + 1) * NB]

                  def zreg(c):
                      ph, i = PH[c]
                      return zt[ph][:, i * NB : (i + 1) * NB]

                  r_s = gt.tile([128, 4, NB], F16, tag="r_s")
                  n_s = gt.tile([128, 4, NB], F16, tag="n_s")
                  hmn_s = gt.tile([128, 4, NB], F16, tag="hmn_s")
                  z_s = gt.tile([128, 4, NB], F16, tag="z_s")
                  t_s = gt.tile([128, 4, NB], F16, tag="t_s")

                  # copy for xt(s+1): emitted at step start so it sits in the
                  # DVE queue right after the previous step's tail, well before
                  # gin0's pt-bank handoff needs it
                  if loop_n:
                      emit_xt_cp((s + 1) % l_steps)
                  elif s >= 1 and s + 1 < l_steps:
                      emit_xt_cp(s + 1)
                  gins = {}

                  def phase_head(ph):
                      ca, cb = (0, 1) if ph == 0 else (2, 3)
                      # one group per bank: start=True only on the bank's first
                      # MM. At s==0 h is zero, the gh matmuls are skipped, and
                      # the head is the whole accumulation (stop here).
                      nc.tensor.matmul(rreg(ca), wih_t[:, ca, :], xt[:], start=True, stop=False)
                      nc.tensor.matmul(rreg(cb), wih_t[:, cb, :], xt[:], start=False, stop=(s == 0))
                      nc.tensor.matmul(nreg(ca), bhhn_t[:, ca * 128 : (ca + 1) * 128],
                                       ones_t[:], start=True, stop=False)
                      nc.tensor.matmul(nreg(cb), bhhn_t[:, cb * 128 : (cb + 1) * 128],
                                       ones_t[:], start=False, stop=(s == 0))

                  def gin_mms(ph):
                      ca, cb = (0, 1) if ph == 0 else (2, 3)
                      if ph == 0:
                          # time-share the pstr bank: ring alternates pt, gin0
                          ps_gin = pstr.tile([128, 2 * NB], F32, tag="pt", name=f"gin{ph}_{s}")
                      else:
                          ps_gin = psgin.tile([128, 2 * NB], F32, tag="gin", name=f"gin{ph}_{s}")
                      gins[ph] = ps_gin
                      nc.tensor.matmul(ps_gin[:, :NB], wih_t[:, 8 + ca, :], xt[:],
                                       start=True, stop=True)
                      nc.tensor.matmul(ps_gin[:, NB:], wih_t[:, 8 + cb, :], xt[:],
                                       start=True, stop=True)

                  def phase_mms(ph):
                      if s == 0:
                          return
                      ca, cb = (0, 1) if ph == 0 else (2, 3)
                      for kt in range(2):
                          last = kt == 1
                          for c in (ca, cb):
                              nc.tensor.matmul(
                                  rreg(c), whh_t[:, kt, :, c * 128 : (c + 1) * 128],
                                  h8_old[:, 2 * kt : 2 * kt + 2, :],
                                  start=False, stop=(last and c == cb), perf_mode=DR)
                          for c in (ca, cb):
                              nc.tensor.matmul(
                                  nreg(c), whh_t[:, kt, :, (8 + c) * 128 : (9 + c) * 128],
                                  h8_old[:, 2 * kt : 2 * kt + 2, :],
                                  start=False, stop=(last and c == cb), perf_mode=DR)

                  def z_mms(ph):
                      ca, cb = (0, 1) if ph == 0 else (2, 3)
                      pool = psza if ph == 0 else pszb
                      zt[ph] = pool.tile([128, 2 * NB], F32, tag=f"z{ph}", name=f"z{ph}_{s}")
                      nc.tensor.matmul(zreg(ca), wih_t[:, 4 + ca, :], xt[:], start=True, stop=False)
                      nc.tensor.matmul(zreg(cb), wih_t[:, 4 + cb, :], xt[:], start=False, stop=(s == 0))
                      if s == 0:
                          return
                      for kt in range(2):
                          last = kt == 1
                          for c in (ca, cb):
                              nc.tensor.matmul(
                                  zreg(c), whh_t[:, kt, :, (4 + c) * 128 : (5 + c) * 128],
                                  h8_old[:, 2 * kt : 2 * kt + 2, :],
                                  start=False, stop=(last and c == cb), perf_mode=DR)

                  sl = lambda ph: slice(2 * ph, 2 * ph + 2)
                  Sigmoid = mybir.ActivationFunctionType.Sigmoid
                  Tanh = mybir.ActivationFunctionType.Tanh

                  # ---- emission order = per-engine queue order. The ACT queue
                  # runs [r0, r1, n0, z02, n1, z13] so the cheap z sigmoids are
                  # not serialized behind the long n chain; the DVE queue runs
                  # [rm0, pre0, rm1, pre1, hmn0, t02, h02, hmn1, t13, h13] so
                  # phase 0's h tail isn't stuck behind phase 1's gate ops.
                  phase_head(0)
                  phase_mms(0)
                  phase_head(1)
                  gin_mms(0)
                  nc.scalar.activation(r_s[:, sl(0), :], r_ab[0][:], Sigmoid, scale=INV)
                  rm0 = gt.tile([128, 2 * NB], F32, tag="rm", name=f"rm0_{s}")
                  nc.vector.tensor_mul(rm0[:], r_s[:, sl(0), :], ghn_ab[0][:])
                  pre0 = gt.tile([128, 2 * NB], F32, tag="pre", name=f"pre0_{s}")
                  nc.vector.tensor_add(pre0[:], rm0[:], gins[0][:])
                  phase_mms(1)
                  nc.scalar.activation(r_s[:, sl(1), :], r_ab[1][:], Sigmoid, scale=INV)
                  gin_mms(1)
                  rm1 = gt.tile([128, 2 * NB], F32, tag="rm", name=f"rm1_{s}")
                  nc.vector.tensor_mul(rm1[:], r_s[:, sl(1), :], ghn_ab[1][:])
                  pre1 = gt.tile([128, 2 * NB], F32, tag="pre", name=f"pre1_{s}")
                  nc.vector.tensor_add(pre1[:], rm1[:], gins[1][:])
                  nc.scalar.activation(n_s[:, sl(0), :], pre0[:], Tanh, scale=INV)
                  z_mms(0)
                  nc.scalar.activation(z_s[:, sl(0), :], zt[0][:], Sigmoid, scale=INV)
                  if s == 0:
                      nc.vector.tensor_scalar_mul(hmn_s[:, sl(0), :], n_s[:, sl(0), :], -1.0)
                  else:
                      nc.vector.tensor_sub(hmn_s[:, sl(0), :], hpair(h_old, 0), n_s[:, sl(0), :])
                  nc.scalar.activation(n_s[:, sl(1), :], pre1[:], Tanh, scale=INV)
                  nc.vector.tensor_mul(t_s[:, sl(0), :], z_s[:, sl(0), :], hmn_s[:, sl(0), :])
                  nc.vector.tensor_add(hpair(h_new, 0), n_s[:, sl(0), :], t_s[:, sl(0), :])
                  nc.scalar.activation(hpair(h8_new, 0), hpair(h_new, 0), Copy)
                  z_mms(1)
                  nc.scalar.activation(z_s[:, sl(1), :], zt[1][:], Sigmoid, scale=INV)
                  if s == 0:
                      nc.vector.tensor_scalar_mul(hmn_s[:, sl(1), :], n_s[:, sl(1), :], -1.0)
                  else:
                      nc.vector.tensor_sub(hmn_s[:, sl(1), :], hpair(h_old, 1), n_s[:, sl(1), :])
                  nc.vector.tensor_mul(t_s[:, sl(1), :], z_s[:, sl(1), :], hmn_s[:, sl(1), :])
                  nc.vector.tensor_add(hpair(h_new, 1), n_s[:, sl(1), :], t_s[:, sl(1), :])
                  nc.scalar.activation(hpair(h8_new, 1), hpair(h_new, 1), Copy)

                  # ---- input prep for step s+2 at the END of the step: the pt
                  # ring then pairs gin0_{s+1} against pt_{s+2} whose freeing
                  # copy has long finished, and the transposes' WAR on pre0_s
                  # lands where the PE is anyway
                  if loop_n:
                      emit_xt_tr((s + 2) % l_steps)
                      emit_gather((s + GA + 1) % l_steps)
                  else:
                      if s + 2 < l_steps:
                          emit_xt_tr(s + 2)
                      if s + GA + 1 < l_steps:
                          emit_gather(s + GA + 1)

            nc.sync.dma_start(out=hout_d[:], in_=h_ring[l_steps % 3][:])

    nc.compile()
    return nc


def build_scores(loop_n=None):
    """128 rows of scores = hc8 @ hr8.T (fp8 DoubleRow) + row softmax, out f16.

    hc8 [p, kt, pl, m] = hc[128*core + m, (2kt+pl)*128 + p]; hr8 likewise over
    all B columns. 8 DR matmuls (K=256 each) accumulate the K=512 contraction.
    """
    from contextlib import nullcontext

    nc = bacc.Bacc("TRN2", target_bir_lowering=False, debug=False)
    hc_d = nc.dram_tensor("hc", [128, 2, 2, 128], F8, kind="ExternalInput")
    hr_d = nc.dram_tensor("hr", [128, 2, 2, B], F8, kind="ExternalInput")
    out_d = nc.dram_tensor("out", [128, B], F16, kind="ExternalOutput")
    NCH = 4
    CW = B // NCH  # 256

    with tile.TileContext(nc) as tc:
        with (
            tc.tile_pool(name="sb", bufs=1) as sb,
            tc.tile_pool(name="ps", bufs=1, space="PSUM") as ps,
            tc.For_i(0, loop_n, 1) if loop_n else nullcontext(),
        ):
            hc_t = sb.tile([128, 2, 2, 128], F8, tag="hc")
            nc.sync.dma_start(out=hc_t[:], in_=hc_d[:])
            hr_t = sb.tile([128, 2, 2, B], F8, tag="hr")
            ps_s = ps.tile([128, B], F32, tag="s")
            for j in range(NCH):
                cs = slice(j * CW, (j + 1) * CW)
                nc.sync.dma_start(out=hr_t[:, :, :, cs], in_=hr_d[:, :, :, cs])
                for kt in range(2):
                    nc.tensor.matmul(
                        ps_s[:, cs], hc_t[:, kt, :, :], hr_t[:, kt, :, cs],
                        start=(kt == 0), stop=(kt == 1), perf_mode=DR)
            ex = sb.tile([128, B], F16, tag="ex")
            ssum = sb.tile([128, 1], F32, tag="ssum")
            nc.scalar.activation(
                ex[:], ps_s[:], mybir.ActivationFunctionType.Exp,
                accum_out=ssum[:])
            rs = sb.tile([128, 1], F32, tag="rs")
            nc.vector.reciprocal(rs[:], ssum[:])
            sm = sb.tile([128, B], F16, tag="sm")
            for hf in range(2):
                hs = slice(hf * 512, (hf + 1) * 512)
                nc.vector.tensor_scalar_mul(sm[:, hs], ex[:, hs], rs[:])
                nc.scalar.dma_start(out=out_d[:, hs], in_=sm[:, hs])

    nc.compile()
    return nc


def _prep_encoder_inputs(tok, emb16, Wih, Whh, bih, bhh):
    """Per-encoder host prep. tok [B, L] int; returns dict pieces shared by its 4 cores."""
    # wih: [E+1, 12, 128]; row E = folded bias (bih+bhh for r,z; bih for n)
    WihT = Wih.T.astype(np.float32)  # [E, 3H]
    brow = np.concatenate([
        (bih[: 2 * H] + bhh[: 2 * H]),
        bih[2 * H :],
    ]).astype(np.float32)  # [3H]
    wih = np.concatenate([WihT, brow[None, :]], axis=0) * S  # [E+1, 3H], x64
    wih = np.ascontiguousarray(
        wih.reshape(E + 1, 12, 128)
    ).astype(np.float16)
    # whh8 [128, kt, pl, 3H]: = Whh[m, (2kt+pl)*128+p] * S, fp8 e4m3
    whh = np.ascontiguousarray(
        np.clip(Whh.T.astype(np.float32) * S, -240, 240).astype(NP8)
        .reshape(2, 2, 128, 3 * H).transpose(2, 0, 1, 3))
    bhhn = (bhh[2 * H :] * S).astype(np.float16)[None, :]  # [1, H], x64
    return wih, whh, bhhn


def _prep_idx(tok_shard):
    """tok_shard [NB, L] -> idx [128, 2L] int32: idx[p, 2s+h] = tok[h*128+p, L-1-s]."""
    t = tok_shard.reshape(2, 128, L)          # [h, p, l]
    rev = t[:, :, ::-1]                        # l -> step s
    idx = rev.transpose(1, 2, 0).reshape(128, L * 2)  # [p, (s, h)]
    return np.ascontiguousarray(idx).astype(np.int32)


_CACHE = {}
TRACE = False
LAST_EXEC_NS = {}


def kernel(contexts, replies, ctx_emb, ctx_Wih, ctx_Whh, ctx_bih, ctx_bhh,
           rep_emb, rep_Wih, rep_Whh, rep_bih, rep_bhh):
    contexts = np.asarray(contexts).astype(np.int32)
    replies = np.asarray(replies).astype(np.int32)
    as32 = lambda a: np.asarray(a, dtype=np.float32)
    ctx_emb16 = as32(ctx_emb).astype(np.float16)
    rep_emb16 = as32(rep_emb).astype(np.float16)

    if "enc" not in _CACHE:
        _CACHE["enc"] = build_encoder()
    if "sco" not in _CACHE:
        _CACHE["sco"] = build_scores()
    enc = _CACHE["enc"]
    sco = _CACHE["sco"]

    cw = _prep_encoder_inputs(contexts, ctx_emb16, as32(ctx_Wih), as32(ctx_Whh),
                              as32(ctx_bih), as32(ctx_bhh))
    rw = _prep_encoder_inputs(replies, rep_emb16, as32(rep_Wih), as32(rep_Whh),
                              as32(rep_bih), as32(rep_bhh))

    in_maps = []
    for c in range(8):
        if c < 4:
            tok, emb16, (wih, whh, bhhn) = contexts, ctx_emb16, cw
            sh = c
        else:
            tok, emb16, (wih, whh, bhhn) = replies, rep_emb16, rw
            sh = c - 4
        in_maps.append({
            "emb": emb16,
            "idx": _prep_idx(tok[sh * NB : (sh + 1) * NB]),
            "wih": wih,
            "whh": whh,
            "bhhn": bhhn,
        })

    res = run_bass_kernel_spmd(enc, in_maps, core_ids=list(range(8)), trace=TRACE)
    if TRACE:
        LAST_EXEC_NS["enc"] = res.exec_time_ns
    houts = [r["hout"] for r in res.results]  # each [128, HC, NB] fp16

    # assemble fp8 score inputs: [ch, p, B] -> [p, kt, pl, B] with ch = 2kt+pl
    hcT = np.concatenate([houts[c].transpose(1, 0, 2) for c in range(4)], axis=2)
    hrT = np.concatenate([houts[c].transpose(1, 0, 2) for c in range(4, 8)], axis=2)
    to8 = lambda a: np.clip(a.astype(np.float32), -240, 240).astype(NP8)
    hc_all = np.ascontiguousarray(
        to8(hcT).reshape(2, 2, 128, B).transpose(2, 0, 1, 3))
    hr_all = np.ascontiguousarray(
        to8(hrT).reshape(2, 2, 128, B).transpose(2, 0, 1, 3))

    in_maps2 = []
    for c in range(8):
        in_maps2.append({
            "hc": np.ascontiguousarray(hc_all[:, :, :, c * 128 : (c + 1) * 128]),
            "hr": hr_all,
        })
    res2 = run_bass_kernel_spmd(sco, in_maps2, core_ids=list(range(8)), trace=TRACE)
    if TRACE:
        LAST_EXEC_NS["sco"] = res2.exec_time_ns
    out = np.concatenate([r["out"] for r in res2.results], axis=0)
    return out.astype(np.float32)



# revision 4
# speedup vs baseline: 2.4816x; 1.1685x over previous
"""DSSM (dual GRU encoder + BxB softmax similarity) on 8 Trainium2 NeuronCores.

Strategy:
  - Cores 0-3 run the context encoder on batch rows [256c, 256(c+1));
    cores 4-7 run the reply encoder on rows [256(c-4), 256(c-3)).
    One SPMD program; per-core in_maps carry the right table/weights/indices.
  - Encoder: backward GRU over L=128 steps, hidden state kept transposed
    (features on partitions): h as [128 part, 4 chunk, 256 batch] fp16.
    Per step: 66 fp16 matmuls (gi K=101 incl. folded biases via ones column,
    gh K=128, bhh_n via K=1 ones outer product) accumulate gate
    pre-activations in PSUM; ACT does sigmoid/tanh; DVE combines.
    Embedding rows arrive via per-step indirect-DMA gathers (+ PE transpose).
  - Phase 2 (second small kernel): each core computes a 128-row slice of
    scores = hc @ hr.T (fp16 matmuls, fp32 accum) + row softmax.

All tensor layout prep (transposes, bias folding, sharding, time reversal)
is host-side numpy; the FLOP-carrying work runs on device.
"""

import numpy as np

import concourse.bass as bass
import concourse.mybir as mybir
import concourse.tile as tile
from concourse import bacc
from concourse.bass_utils import run_bass_kernel_spmd
from concourse.masks import make_identity

F16 = mybir.dt.float16
F32 = mybir.dt.float32
I32 = mybir.dt.int32
F8 = mybir.dt.float8e4
NP8 = mybir.dt.np(F8)
DR = mybir.MatmulPerfMode.DoubleRow
Copy = mybir.ActivationFunctionType.Copy
S = 64.0
INV = float(1.0 / S)

V, E, H, B, L = 50000, 100, 512, 1024, 128
NB = 256          # batch rows per core
HC = H // 128     # 4 h chunks
GA = 6            # gather lookahead (steps)
XA = 2            # xt lookahead


def build_encoder(l_steps=L, loop_n=None):
    """loop_n: benchmark-only mode — wraps the step loop in a For_i that
    re-runs the whole sequence loop_n times (data goes stale; timing only)."""
    nc = bacc.Bacc("TRN2", target_bir_lowering=False, debug=False)
    emb_d = nc.dram_tensor("emb", [V, E], F16, kind="ExternalInput")
    idx_d = nc.dram_tensor("idx", [128, 2 * l_steps], I32, kind="ExternalInput")
    wih_d = nc.dram_tensor("wih", [E + 1, 12, 128], F16, kind="ExternalInput")
    whh_d = nc.dram_tensor("whh", [128, 2, 2, 3 * H], F8, kind="ExternalInput")
    bhhn_d = nc.dram_tensor("bhhn", [1, H], F16, kind="ExternalInput")
    hout_d = nc.dram_tensor("hout", [128, HC, NB], F16, kind="ExternalOutput")

    with tile.TileContext(nc) as tc:
        with (
            tc.tile_pool(name="wt", bufs=1) as wt,
            tc.tile_pool(name="grng", bufs=GA + 2) as grng,
            tc.tile_pool(name="xrng", bufs=XA + 2) as xrng,
            tc.tile_pool(name="hrng", bufs=3) as hrng,
            tc.tile_pool(name="gt", bufs=4) as gt,
            # one PSUM pool per bank; pstr also hosts gin0 (time-shared slot)
            tc.tile_pool(name="psra", bufs=1, space="PSUM") as psra,
            tc.tile_pool(name="psrb", bufs=1, space="PSUM") as psrb,
            tc.tile_pool(name="psza", bufs=1, space="PSUM") as psza,
            tc.tile_pool(name="pszb", bufs=1, space="PSUM") as pszb,
            tc.tile_pool(name="psga", bufs=1, space="PSUM") as psga,
            tc.tile_pool(name="psgb", bufs=1, space="PSUM") as psgb,
            tc.tile_pool(name="psgin", bufs=1, space="PSUM") as psgin,
            tc.tile_pool(name="pstr", bufs=1, space="PSUM") as pstr,
        ):
            # --- constants / weights (idx first: gathers need it; whh last:
            # step 0 skips its gh matmuls since h0 == 0, so whh is only
            # needed from step 1) ---
            idx_t = wt.tile([128, 2 * l_steps], I32, tag="idx")
            nc.sync.dma_start(out=idx_t[:], in_=idx_d[:])
            wih_t = wt.tile([E + 1, 12, 128], F16, tag="wih")
            nc.sync.dma_start(out=wih_t[:], in_=wih_d[:])
            bhhn_t = wt.tile([1, H], F16, tag="bhhn")
            nc.sync.dma_start(out=bhhn_t[:], in_=bhhn_d[:])
            # whh split r/n/z in consumption order: step 1's r matmuls can
            # start ~3us before the z columns finish loading
            whh_t = wt.tile([128, 2, 2, 3 * H], F8, tag="whh")
            nc.sync.dma_start(out=whh_t[:, :, :, 0 : H], in_=whh_d[:, :, :, 0 : H])
            nc.sync.dma_start(out=whh_t[:, :, :, 2 * H : 3 * H], in_=whh_d[:, :, :, 2 * H : 3 * H])
            nc.sync.dma_start(out=whh_t[:, :, :, H : 2 * H], in_=whh_d[:, :, :, H : 2 * H])
            ident = wt.tile([128, 128], F16, tag="ident")
            make_identity(nc, ident[:])
            ones_t = wt.tile([1, NB], F16, tag="ones")
            nc.vector.memset(ones_t[:], 1.0)

            # --- rings ---
            n_g = GA + 2
            g_ring = []
            for i in range(n_g):
                g = grng.tile([128, 2, E + 1], F16, tag="g", name=f"g{i}")
                nc.vector.memset(g[:, :, E : E + 1], 1.0)
                g_ring.append(g)
            n_x = XA + 2
            xt_ring = [xrng.tile([E + 1, NB], F16, tag="xt", name=f"xt{i}") for i in range(n_x)]
            # no h memsets: step 0 skips all gh matmuls and computes its hmn
            # as -n directly, so no slot is ever read before it is written
            h_ring = [hrng.tile([128, HC, NB], F16, tag="h", name=f"h{i}")
                      for i in range(3)]
            h8_ring = [hrng.tile([128, HC, NB], F8, tag="h8", name=f"h8{i}")
                       for i in range(3)]

            def emit_gather(s):
                g = g_ring[s % n_g]
                for hh in range(2):
                    nc.gpsimd.indirect_dma_start(
                        out=g[:, hh, :E],
                        out_offset=None,
                        in_=emb_d[:],
                        in_offset=bass.IndirectOffsetOnAxis(
                            ap=idx_t[:, 2 * s + hh : 2 * s + hh + 1], axis=0
                        ),
                    )

            pts = {}

            def emit_xt_tr(s):
                g = g_ring[s % n_g]
                pt = pstr.tile([E + 1, 2, 128], F16, tag="pt", name=f"pt{s}")
                pts[s % n_x] = pt
                for hh in range(2):
                    nc.tensor.transpose(pt[:, hh, :], g[:, hh, :], ident[:])

            def emit_xt_cp(s):
                xt = xt_ring[s % n_x]
                pt = pts[s % n_x]
                nc.vector.tensor_copy(xt[:, :], pt.rearrange("p a b -> p (a b)"))

            # prologue
            for s in range(min(GA + 1, l_steps)):
                emit_gather(s)
            emit_xt_tr(0)
            emit_xt_cp(0)
            if l_steps > 1:
                emit_xt_tr(1)
                emit_xt_cp(1)

            # chunk pairs: phase 0 handles chunks (0,2), phase 1 chunks (1,3).
            # kj-major matmul order [0,2,1,3] matches the order h chunks are
            # produced by the previous step's tail, so the PE never waits for
            # the full h vector — only for the chunk its current MM reads.
            # PSUM region order is phase-major [c0, c2, c1, c3]: each phase
            # owns whole banks, with ONE accumulation group per bank (start
            # clears has_written bank-wide once; later first-writes to other
            # regions overwrite because their bits are cleared too).
            def hpair(h, b):
                return h[:, 2 * b : 2 * b + 2, :]

            from contextlib import nullcontext
            loop_cm = tc.For_i(0, loop_n, 1) if loop_n else nullcontext()
            with loop_cm:
              for s in range(l_steps):
                  h_old = h_ring[s % 3]
                  h_new = h_ring[(s + 1) % 3]
                  h8_old = h8_ring[s % 3]
                  h8_new = h8_ring[(s + 1) % 3]
                  xt = xt_ring[s % n_x]

                  # per-phase PSUM tiles (one bank each) so cross-phase ops on
                  # different banks never serialize on tile-level hazards
                  r_ab = [psra.tile([128, 2 * NB], F32, tag="ra", name=f"ra_{s}"),
                          psrb.tile([128, 2 * NB], F32, tag="rb", name=f"rb_{s}")]
                  ghn_ab = [psga.tile([128, 2 * NB], F32, tag="ga", name=f"ga_{s}"),
                            psgb.tile([128, 2 * NB], F32, tag="gb", name=f"gb_{s}")]
                  zt = {}
                  PH = {0: (0, 0), 1: (0, 1), 2: (1, 0), 3: (1, 1)}

                  def rreg(c):
                      ph, i = PH[c]
                      return r_ab[ph][:, i * NB : (i + 1) * NB]

                  def nreg(c):
                      ph, i = PH[c]
                      return ghn_ab[ph][:, i * NB : (i + 1) * NB]

                  def zreg(c):
                      ph, i = PH[c]
                      return zt[ph][:, i * NB : (i + 1) * NB]

                  r_s = gt.tile([128, 4, NB], F16, tag="r_s")
                  gs0 = gt.tile([128, 2 * NB], F16, tag="gs0", name=f"gs0_{s}")
                  gs1 = gt.tile([128, 2 * NB], F16, tag="gs1", name=f"gs1_{s}")
                  n_s = gt.tile([128, 4, NB], F16, tag="n_s")
                  hmn_s = gt.tile([128, 4, NB], F16, tag="hmn_s")
                  z_s = gt.tile([128, 4, NB], F16, tag="z_s")
                  t_s = gt.tile([128, 4, NB], F16, tag="t_s")

                  # copy for xt(s+1): emitted at step start so it sits in the
                  # DVE queue right after the previous step's tail, well before
                  # gin0's pt-bank handoff needs it
                  if loop_n:
                      emit_xt_cp((s + 1) % l_steps)
                  elif s >= 1 and s + 1 < l_steps:
                      emit_xt_cp(s + 1)
                  gins = {}

                  def phase_head(ph):
                      ca, cb = (0, 1) if ph == 0 else (2, 3)
                      # one group per bank: start=True only on the bank's first
                      # MM. At s==0 h is zero, the gh matmuls are skipped, and
                      # the head is the whole accumulation (stop here).
                      nc.tensor.matmul(rreg(ca), wih_t[:, ca, :], xt[:], start=True, stop=False)
                      nc.tensor.matmul(rreg(cb), wih_t[:, cb, :], xt[:], start=False, stop=(s == 0))
                      nc.tensor.matmul(nreg(ca), bhhn_t[:, ca * 128 : (ca + 1) * 128],
                                       ones_t[:], start=True, stop=False)
                      nc.tensor.matmul(nreg(cb), bhhn_t[:, cb * 128 : (cb + 1) * 128],
                                       ones_t[:], start=False, stop=(s == 0))

                  def gin_mms(ph):
                      ca, cb = (0, 1) if ph == 0 else (2, 3)
                      if ph == 0:
                          # time-share the pstr bank: ring alternates pt, gin0
                          ps_gin = pstr.tile([128, 2 * NB], F32, tag="pt", name=f"gin{ph}_{s}")
                      else:
                          ps_gin = psgin.tile([128, 2 * NB], F32, tag="gin", name=f"gin{ph}_{s}")
                      gins[ph] = ps_gin
                      nc.tensor.matmul(ps_gin[:, :NB], wih_t[:, 8 + ca, :], xt[:],
                                       start=True, stop=True)
                      nc.tensor.matmul(ps_gin[:, NB:], wih_t[:, 8 + cb, :], xt[:],
                                       start=True, stop=True)

                  def phase_mms(ph):
                      if s == 0:
                          return
                      ca, cb = (0, 1) if ph == 0 else (2, 3)
                      for kt in range(2):
                          last = kt == 1
                          for c in (ca, cb):
                              nc.tensor.matmul(
                                  rreg(c), whh_t[:, kt, :, c * 128 : (c + 1) * 128],
                                  h8_old[:, 2 * kt : 2 * kt + 2, :],
                                  start=False, stop=(last and c == cb), perf_mode=DR)
                          for c in (ca, cb):
                              nc.tensor.matmul(
                                  nreg(c), whh_t[:, kt, :, (8 + c) * 128 : (9 + c) * 128],
                                  h8_old[:, 2 * kt : 2 * kt + 2, :],
                                  start=False, stop=(last and c == cb), perf_mode=DR)

                  def z_mms(ph):
                      ca, cb = (0, 1) if ph == 0 else (2, 3)
                      pool = psza if ph == 0 else pszb
                      zt[ph] = pool.tile([128, 2 * NB], F32, tag=f"z{ph}", name=f"z{ph}_{s}")
                      nc.tensor.matmul(zreg(ca), wih_t[:, 4 + ca, :], xt[:], start=True, stop=False)
                      nc.tensor.matmul(zreg(cb), wih_t[:, 4 + cb, :], xt[:], start=False, stop=(s == 0))
                      if s == 0:
                          return
                      for kt in range(2):
                          last = kt == 1
                          for c in (ca, cb):
                              nc.tensor.matmul(
                                  zreg(c), whh_t[:, kt, :, (4 + c) * 128 : (5 + c) * 128],
                                  h8_old[:, 2 * kt : 2 * kt + 2, :],
                                  start=False, stop=(last and c == cb), perf_mode=DR)

                  sl = lambda ph: slice(2 * ph, 2 * ph + 2)
                  Sigmoid = mybir.ActivationFunctionType.Sigmoid
                  Tanh = mybir.ActivationFunctionType.Tanh

                  # ---- emission order = per-engine queue order. The ACT queue
                  # runs [r0, r1, n0, z02, n1, z13] so the cheap z sigmoids are
                  # not serialized behind the long n chain; the DVE queue runs
                  # [rm0, pre0, rm1, pre1, hmn0, t02, h02, hmn1, t13, h13] so
                  # phase 0's h tail isn't stuck behind phase 1's gate ops.
                  phase_head(0)
                  phase_mms(0)
                  phase_head(1)
                  gin_mms(0)
                  nc.scalar.activation(r_s[:, sl(0), :], r_ab[0][:], Sigmoid, scale=INV)
                  nc.scalar.activation(gs0[:], ghn_ab[0][:], Copy)
                  rm0 = gt.tile([128, 2 * NB], F16, tag="rm", name=f"rm0_{s}")
                  nc.vector.tensor_mul(rm0[:], r_s[:, sl(0), :], gs0[:])
                  pre0 = gt.tile([128, 2 * NB], F32, tag="pre", name=f"pre0_{s}")
                  nc.vector.tensor_add(pre0[:], rm0[:], gins[0][:])
                  phase_mms(1)
                  nc.scalar.activation(r_s[:, sl(1), :], r_ab[1][:], Sigmoid, scale=INV)
                  nc.scalar.activation(gs1[:], ghn_ab[1][:], Copy)
                  gin_mms(1)
                  rm1 = gt.tile([128, 2 * NB], F16, tag="rm", name=f"rm1_{s}")
                  nc.vector.tensor_mul(rm1[:], r_s[:, sl(1), :], gs1[:])
                  pre1 = gt.tile([128, 2 * NB], F32, tag="pre", name=f"pre1_{s}")
                  nc.vector.tensor_add(pre1[:], rm1[:], gins[1][:])
                  nc.scalar.activation(n_s[:, sl(0), :], pre0[:], Tanh, scale=INV)
                  z_mms(0)
                  nc.scalar.activation(z_s[:, sl(0), :], zt[0][:], Sigmoid, scale=INV)
                  if s == 0:
                      nc.vector.tensor_scalar_mul(hmn_s[:, sl(0), :], n_s[:, sl(0), :], -1.0)
                  else:
                      nc.vector.tensor_sub(hmn_s[:, sl(0), :], hpair(h_old, 0), n_s[:, sl(0), :])
                  nc.scalar.activation(n_s[:, sl(1), :], pre1[:], Tanh, scale=INV)
                  nc.vector.tensor_mul(t_s[:, sl(0), :], z_s[:, sl(0), :], hmn_s[:, sl(0), :])
                  nc.vector.tensor_add(hpair(h_new, 0), n_s[:, sl(0), :], t_s[:, sl(0), :])
                  nc.scalar.activation(hpair(h8_new, 0), hpair(h_new, 0), Copy)
                  z_mms(1)
                  nc.scalar.activation(z_s[:, sl(1), :], zt[1][:], Sigmoid, scale=INV)
                  if s == 0:
                      nc.vector.tensor_scalar_mul(hmn_s[:, sl(1), :], n_s[:, sl(1), :], -1.0)
                  else:
                      nc.vector.tensor_sub(hmn_s[:, sl(1), :], hpair(h_old, 1), n_s[:, sl(1), :])
                  nc.vector.tensor_mul(t_s[:, sl(1), :], z_s[:, sl(1), :], hmn_s[:, sl(1), :])
                  nc.vector.tensor_add(hpair(h_new, 1), n_s[:, sl(1), :], t_s[:, sl(1), :])
                  nc.scalar.activation(hpair(h8_new, 1), hpair(h_new, 1), Copy)

                  # ---- input prep for step s+2 at the END of the step: the pt
                  # ring then pairs gin0_{s+1} against pt_{s+2} whose freeing
                  # copy has long finished, and the transposes' WAR on pre0_s
                  # lands where the PE is anyway
                  if loop_n:
                      emit_xt_tr((s + 2) % l_steps)
                      emit_gather((s + GA + 1) % l_steps)
                  else:
                      if s + 2 < l_steps:
                          emit_xt_tr(s + 2)
                      if s + GA + 1 < l_steps:
                          emit_gather(s + GA + 1)

            nc.sync.dma_start(out=hout_d[:], in_=h_ring[l_steps % 3][:])

    nc.compile()
    return nc


def build_scores(loop_n=None):
    """128 rows of scores = hc8 @ hr8.T (fp8 DoubleRow) + row softmax, out f16.

    hc8 [p, kt, pl, m] = hc[128*core + m, (2kt+pl)*128 + p]; hr8 likewise over
    all B columns. 8 DR matmuls (K=256 each) accumulate the K=512 contraction.
    """
    from contextlib import nullcontext

    nc = bacc.Bacc("TRN2", target_bir_lowering=False, debug=False)
    hc_d = nc.dram_tensor("hc", [128, 2, 2, 128], F8, kind="ExternalInput")
    hr_d = nc.dram_tensor("hr", [128, 2, 2, B], F8, kind="ExternalInput")
    out_d = nc.dram_tensor("out", [128, B], F16, kind="ExternalOutput")
    NCH = 4
    CW = B // NCH  # 256

    with tile.TileContext(nc) as tc:
        with (
            tc.tile_pool(name="sb", bufs=1) as sb,
            tc.tile_pool(name="ps", bufs=1, space="PSUM") as ps,
            tc.For_i(0, loop_n, 1) if loop_n else nullcontext(),
        ):
            hc_t = sb.tile([128, 2, 2, 128], F8, tag="hc")
            nc.sync.dma_start(out=hc_t[:], in_=hc_d[:])
            hr_t = sb.tile([128, 2, 2, B], F8, tag="hr")
            ps_s = ps.tile([128, B], F32, tag="s")
            for j in range(NCH):
                cs = slice(j * CW, (j + 1) * CW)
                nc.sync.dma_start(out=hr_t[:, :, :, cs], in_=hr_d[:, :, :, cs])
                for kt in range(2):
                    nc.tensor.matmul(
                        ps_s[:, cs], hc_t[:, kt, :, :], hr_t[:, kt, :, cs],
                        start=(kt == 0), stop=(kt == 1), perf_mode=DR)
            ex = sb.tile([128, B], F16, tag="ex")
            ssum = sb.tile([128, 1], F32, tag="ssum")
            nc.scalar.activation(
                ex[:], ps_s[:], mybir.ActivationFunctionType.Exp,
                accum_out=ssum[:])
            rs = sb.tile([128, 1], F32, tag="rs")
            nc.vector.reciprocal(rs[:], ssum[:])
            sm = sb.tile([128, B], F16, tag="sm")
            for hf in range(2):
                hs = slice(hf * 512, (hf + 1) * 512)
                nc.vector.tensor_scalar_mul(sm[:, hs], ex[:, hs], rs[:])
                nc.scalar.dma_start(out=out_d[:, hs], in_=sm[:, hs])

    nc.compile()
    return nc


def _prep_encoder_inputs(tok, emb16, Wih, Whh, bih, bhh):
    """Per-encoder host prep. tok [B, L] int; returns dict pieces shared by its 4 cores."""
    # wih: [E+1, 12, 128]; row E = folded bias (bih+bhh for r,z; bih for n)
    WihT = Wih.T.astype(np.float32)  # [E, 3H]
    brow = np.concatenate([
        (bih[: 2 * H] + bhh[: 2 * H]),
        bih[2 * H :],
    ]).astype(np.float32)  # [3H]
    wih = np.concatenate([WihT, brow[None, :]], axis=0) * S  # [E+1, 3H], x64
    wih = np.ascontiguousarray(
        wih.reshape(E + 1, 12, 128)
    ).astype(np.float16)
    # whh8 [128, kt, pl, 3H]: = Whh[m, (2kt+pl)*128+p] * S, fp8 e4m3
    whh = np.ascontiguousarray(
        np.clip(Whh.T.astype(np.float32) * S, -240, 240).astype(NP8)
        .reshape(2, 2, 128, 3 * H).transpose(2, 0, 1, 3))
    bhhn = (bhh[2 * H :] * S).astype(np.float16)[None, :]  # [1, H], x64
    return wih, whh, bhhn


def _prep_idx(tok_shard):
    """tok_shard [NB, L] -> idx [128, 2L] int32: idx[p, 2s+h] = tok[h*128+p, L-1-s]."""
    t = tok_shard.reshape(2, 128, L)          # [h, p, l]
    rev = t[:, :, ::-1]                        # l -> step s
    idx = rev.transpose(1, 2, 0).reshape(128, L * 2)  # [p, (s, h)]
    return np.ascontiguousarray(idx).astype(np.int32)


_CACHE = {}
TRACE = False
LAST_EXEC_NS = {}


def kernel(contexts, replies, ctx_emb, ctx_Wih, ctx_Whh, ctx_bih, ctx_bhh,
           rep_emb, rep_Wih, rep_Whh, rep_bih, rep_bhh):
    contexts = np.asarray(contexts).astype(np.int32)
    replies = np.asarray(replies).astype(np.int32)
    as32 = lambda a: np.asarray(a, dtype=np.float32)
    ctx_emb16 = as32(ctx_emb).astype(np.float16)
    rep_emb16 = as32(rep_emb).astype(np.float16)

    if "enc" not in _CACHE:
        _CACHE["enc"] = build_encoder()
    if "sco" not in _CACHE:
        _CACHE["sco"] = build_scores()
    enc = _CACHE["enc"]
    sco = _CACHE["sco"]

    cw = _prep_encoder_inputs(contexts, ctx_emb16, as32(ctx_Wih), as32(ctx_Whh),
                              as32(ctx_bih), as32(ctx_bhh))
    rw = _prep_encoder_inputs(replies, rep_emb16, as32(rep_Wih), as32(rep_Whh),
                              as32(rep_bih), as32(rep_bhh))

    in_maps = []
    for c in range(8):
        if c < 4:
            tok, emb16, (wih, whh, bhhn) = contexts, ctx_emb16, cw
            sh = c
        else:
            tok, emb16, (wih, whh, bhhn) = replies, rep_emb16, rw
            sh = c - 4
        in_maps.append({
            "emb": emb16,
            "idx": _prep_idx(tok[sh * NB : (sh + 1) * NB]),
            "wih": wih,
            "whh": whh,
            "bhhn": bhhn,
        })

    res = run_bass_kernel_spmd(enc, in_maps, core_ids=list(range(8)), trace=TRACE)
    if TRACE:
        LAST_EXEC_NS["enc"] = res.exec_time_ns
    houts = [r["hout"] for r in res.results]  # each [128, HC, NB] fp16

    # assemble fp8 score inputs: [ch, p, B] -> [p, kt, pl, B] with ch = 2kt+pl
    hcT = np.concatenate([houts[c].transpose(1, 0, 2) for c in range(4)], axis=2)
    hrT = np.concatenate([houts[c].transpose(1, 0, 2) for c in range(4, 8)], axis=2)
    to8 = lambda a: np.clip(a.astype(np.float32), -240, 240).astype(NP8)
    hc_all = np.ascontiguousarray(
        to8(hcT).reshape(2, 2, 128, B).transpose(2, 0, 1, 3))
    hr_all = np.ascontiguousarray(
        to8(hrT).reshape(2, 2, 128, B).transpose(2, 0, 1, 3))

    in_maps2 = []
    for c in range(8):
        in_maps2.append({
            "hc": np.ascontiguousarray(hc_all[:, :, :, c * 128 : (c + 1) * 128]),
            "hr": hr_all,
        })
    res2 = run_bass_kernel_spmd(sco, in_maps2, core_ids=list(range(8)), trace=TRACE)
    if TRACE:
        LAST_EXEC_NS["sco"] = res2.exec_time_ns
    out = np.concatenate([r["out"] for r in res2.results], axis=0)
    return out.astype(np.float32)

